# revision 58
# baseline (speedup 1.0000x reference)
"""EncoderDecoderRNN (GRU enc/dec + per-step linear + log_softmax) on 8 trn2 cores.

Data-parallel: batch 256 -> 32 per core. Per core, everything lives in a
"G-layout": a [B=32, D=512] tensor is stored as SBUF [128, 128] with
partition 32*g + b holding d-slice [128g, 128g+128) for sample b. This puts
all 128 partitions to work on elementwise ops (free-dim 128 instead of 512).

Key tricks:
- gi = x @ Wih.T is a table lookup: EmbWih = emb @ Wih.T is [V=512, 1536],
  so per step gi_t[b] = EmbWih[src[b,t]] -- one indirect-DMA gather per step
  from a DRAM table laid out so the gather lands directly in G-layout.
- The recurrent gh = h @ Whh.T runs as 16 matmuls per step: stationary
  h.T chunks [128, 32] at 4 tile-position col-groups (concurrent in the PE
  array), 4 K-chunks PSUM-accumulated, float32r for single-pass fp32 speed.
- Decoder logits (h @ out_W.T) stream as a 4th 128-col block in the same
  matmuls (Whh-stream and outW-stream concatenated in SBUF).
- h.T for the next step's stationary comes from DVE 32x32 block transposes
  (the d-order inside each K-chunk is a fixed permutation, pre-applied to
  the streamed weight rows on the host).
- log_softmax runs on flat [128=4 steps x 32 b, 512] tiles every 4 decoder
  steps: -max via negated reduce, exp with accumulate for the sum, final
  subtract fused into an Identity-activation with per-partition bias.
"""

import numpy as np
from contextlib import ExitStack

import concourse.bass as bass
import concourse.mybir as mybir
import concourse.tile as tile
from concourse.bass_utils import run_bass_kernel_spmd

F32 = mybir.dt.float32
F32R = mybir.dt.float32r
BF16 = mybir.dt.bfloat16
I32 = mybir.dt.int32
AF = mybir.ActivationFunctionType
ALU = mybir.AluOpType

B, S, T, V, D = 256, 256, 256, 512, 512
NCORES = 8
BL = B // NCORES          # 32 samples per core
G = 4                     # d-groups (128 each) on partitions
NK = 4                    # K-chunks of 128 for the D=512 contraction
TD3 = 3 * D               # 1536


def _split_waits(nc, max_waits=1):
    """Walrus in this container accepts at most one sync-wait per instruction;
    split extras into preceding same-engine NoOps."""
    n = 0
    for fn in nc.m.functions:
        for block in fn.blocks:
            out, changed = [], False
            for inst in block.instructions:
                si = inst.sync_info
                waits = list(si.on_wait) if si is not None else []
                if len(waits) > max_waits:
                    changed = True
                    keep = waits[-max_waits:]
                    extra = waits[:-max_waits]
                    for w in extra:
                        n += 1
                        out.append(mybir.InstNoOp(
                            name=f"waitsplit-{n}", engine=inst.engine,
                            ins=[], outs=[],
                            sync_info=mybir.SyncInfo(on_wait=[w], on_update=[])))
                    inst.sync_info = mybir.SyncInfo(
                        on_wait=keep, on_update=list(si.on_update))
                out.append(inst)
            if changed:
                block.instructions = out


def build_program(ss=S, tt=T, split=True, dbg=False):
    """Build the per-core bass program (same on all cores; data differs)."""
    nc = bass.Bass("TRN2", target_bir_lowering=False, debug=False,
                   num_devices=NCORES)

    ngrp = tt // 4
    # ---- DRAM I/O (host-prepped layouts; see host_prep) ----
    # gather indices, pre-offset: gidx[32g+b, t] = g*V + tok[b, t]
    d_gidx_s = nc.dram_tensor("gidx_src", [128, ss], I32, kind="ExternalInput")
    d_gidx_t = nc.dram_tensor("gidx_trg", [128, tt], I32, kind="ExternalInput")
    d_ident = nc.dram_tensor("ident", [128, 128], BF16, kind="ExternalInput")
    # lhsT chunks for the EmbWih builds: embT[c, p, v] = emb[v, 128c+p]
    d_embT_e = nc.dram_tensor("embT_enc", [NK, 128, V], BF16, kind="ExternalInput")
    d_embT_d = nc.dram_tensor("embT_dec", [NK, 128, V], BF16, kind="ExternalInput")
    # rhs chunks: wihT[c, p, q] = Wih[q, 128c+p]
    d_wihT_e = nc.dram_tensor("wihT_enc", [NK, 128, TD3], BF16, kind="ExternalInput")
    d_wihT_d = nc.dram_tensor("wihT_dec", [NK, 128, TD3], BF16, kind="ExternalInput")
    # recurrent streams, Dk-permuted rows; dec has outW columns appended
    d_whh_e = nc.dram_tensor("whh_enc", [NK, 128, TD3], BF16, kind="ExternalInput")
    d_whh_d = nc.dram_tensor("whh_dec", [NK, 128, TD3 + D], BF16, kind="ExternalInput")
    # flat output: out[j, 32s+b, v] = log_softmax(b, 4j+s, v); host reorders
    d_out = nc.dram_tensor("out", [ngrp, 128, V], F32, kind="ExternalOutput")
    # gather tables (device-computed): tab[g*V + v, m*128 + j] = EmbW[v, m*512 + 128g + j]
    d_tab_e = nc.dram_tensor("tab_enc", [G * V, 384], F32, kind="Internal")
    d_tab_d = nc.dram_tensor("tab_dec", [G * V, 384], F32, kind="Internal")
    if dbg:
        d_dbg_h = nc.dram_tensor("dbg_h", [128, 128], F32, kind="ExternalOutput")
        d_dbg_fl = nc.dram_tensor("dbg_fl", [128, 512], F32, kind="ExternalOutput")
        d_dbg_gi = nc.dram_tensor("dbg_gi", [128, 384], F32, kind="ExternalOutput")
        d_dbg_ps = nc.dram_tensor("dbg_ps", [128, 512], F32, kind="ExternalOutput")
        d_dbg_w = {}
        for wn in ["rp", "r", "t1", "t2", "n", "zp", "z", "zb", "hz", "nzb", "gi0", "ps0", "hT0", "ps1"]:
            wid = 384 if wn == "gi0" else (512 if wn in ("ps0", "ps1") else 128)
            d_dbg_w[wn] = nc.dram_tensor(f"dbgw_{wn}", [128, wid], F32, kind="ExternalOutput")

    with tile.TileContext(nc) as tc:
        with ExitStack() as ctx:
            singles = ctx.enter_context(tc.tile_pool(name="singles", bufs=1))

            # ---- persistent SBUF ----
            whh_e = singles.tile([128, NK, TD3], BF16)
            nc.sync.dma_start(whh_e, d_whh_e.ap().rearrange("k p q -> p k q"))
            whh_d = singles.tile([128, NK, TD3 + D], BF16)
            nc.sync.dma_start(whh_d, d_whh_d.ap().rearrange("k p q -> p k q"))

            gidx_s = singles.tile([128, ss], I32)
            nc.sync.dma_start(gidx_s, d_gidx_s.ap())
            gidx_t = singles.tile([128, tt], I32)
            nc.sync.dma_start(gidx_t, d_gidx_t.ap())
            ident = singles.tile([128, 128], BF16)
            nc.sync.dma_start(ident, d_ident.ap())
            # raw logits, flat: raw[32s+b, j, v] = logit(b, 4j+s, v)
            raw = singles.tile([128, tt // 4, V], BF16)

            # ---- build the two gather tables on-device ----
            def build_table(d_embT, d_wihT, d_tab, do_relu):
                with ExitStack() as sctx:
                    setup = sctx.enter_context(tc.tile_pool(name="setup", bufs=1))
                    spsum = sctx.enter_context(
                        tc.tile_pool(name="spsum", bufs=2, space="PSUM"))
                    embT = setup.tile([128, NK, V], BF16)
                    nc.sync.dma_start(embT, d_embT.ap().rearrange("c p v -> p c v"))
                    wihT = setup.tile([128, NK, TD3], BF16)
                    nc.sync.dma_start(wihT, d_wihT.ap().rearrange("c p q -> p c q"))
                    if do_relu:
                        nc.scalar.activation(embT, embT, AF.Relu)
                    for vt in range(V // 128):
                        ps = spsum.tile([128, TD3], F32, tag="embw")
                        for c in range(NK):
                            for nb in range(TD3 // 512):
                                nc.tensor.matmul(
                                    ps[:, 512 * nb:512 * nb + 512],
                                    lhsT=embT[:, c, 128 * vt:128 * vt + 128],
                                    rhs=wihT[:, c, 512 * nb:512 * nb + 512],
                                    start=(c == 0), stop=(c == NK - 1))
                        stage = setup.tile([128, TD3], F32, tag="stage")
                        # cols (m*512 + 128g + j) -> (g, m, j)
                        src = ps[:, :].rearrange("p (m g j) -> p g m j", m=3, g=G)
                        dst = stage[:, :].rearrange("p (g m j) -> p g m j", m=3, g=G)
                        nc.scalar.copy(dst, src)
                        st3 = stage[:, :].rearrange("p (g x) -> p g x", g=G)
                        for g in range(G):
                            nc.sync.dma_start(
                                d_tab.ap()[g * V + 128 * vt: g * V + 128 * vt + 128, :],
                                st3[:, g, :])

            build_table(d_embT_e, d_wihT_e, d_tab_e, do_relu=False)
            build_table(d_embT_d, d_wihT_d, d_tab_d, do_relu=True)

            # ---- loop pools ----
            gip = ctx.enter_context(tc.tile_pool(name="gi", bufs=8))
            psp = ctx.enter_context(tc.tile_pool(name="ps", bufs=4, space="PSUM"))
            fpp = ctx.enter_context(tc.tile_pool(name="fp", bufs=2, space="PSUM"))
            wk = ctx.enter_context(tc.tile_pool(name="wk", bufs=3))
            hp = ctx.enter_context(tc.tile_pool(name="h", bufs=2))
            fl = ctx.enter_context(tc.tile_pool(name="fl", bufs=2))

            h_G = hp.tile([128, 128], F32, tag="hG")
            hT = hp.tile([128, 128], BF16, tag="hT")
            nc.vector.memset(h_G, 0.0)
            nc.vector.memset(hT, 0.0)

            def phase(steps, whh, gidx, tab, is_dec):
                nonlocal h_G, hT
                mw = 4 if is_dec else 3          # streamed 512-blocks per MM
                flat = None
                for t in range(steps):
                    gi = gip.tile([128, 384], F32, tag="gi")
                    nc.gpsimd.indirect_dma_start(
                        out=gi[:, :], out_offset=None, in_=tab.ap(),
                        in_offset=bass.IndirectOffsetOnAxis(ap=gidx[:, t:t + 1], axis=0))
                    ps = psp.tile([128, 512], F32, tag="ps")
                    for k in range(NK):
                        for g in range(G):
                            rhs = whh[:, k, :].rearrange(
                                "p (m x) -> p m x", x=512)[:, 0:mw, 128 * g:128 * g + 128]
                            out = ps[32 * g:32 * g + 32, 0:mw * 128].rearrange(
                                "p (m x) -> p m x", x=128)
                            nc.tensor.matmul(
                                out, lhsT=hT[:, 32 * k:32 * k + 32],
                                rhs=rhs,
                                start=(k == 0), stop=(k == NK - 1),
                                tile_position=(0, 32 * g),
                                skip_group_check=True)
                    # gates; cols [r | z | gh_n (| logits)] of 128 each
                    rp = wk.tile([128, 128], F32, tag="rp")
                    nc.vector.tensor_add(rp, gi[:, 0:128], ps[:, 0:128])
                    r = wk.tile([128, 128], F32, tag="r")
                    nc.scalar.activation(r, rp, AF.Sigmoid)
                    t1 = wk.tile([128, 128], F32, tag="t1")
                    nc.vector.tensor_mul(t1, r, ps[:, 256:384])
                    t2 = wk.tile([128, 128], F32, tag="t2")
                    nc.vector.tensor_add(t2, t1, gi[:, 256:384])
                    n_ = wk.tile([128, 128], F32, tag="n")
                    nc.scalar.activation(n_, t2, AF.Tanh)

                    zp = wk.tile([128, 128], F32, tag="zp")
                    nc.vector.tensor_add(zp, gi[:, 128:256], ps[:, 128:256])
                    z = wk.tile([128, 128], F32, tag="z")
                    nc.scalar.activation(z, zp, AF.Sigmoid)
                    zb = wk.tile([128, 128], F32, tag="zb")
                    nc.vector.tensor_scalar(zb, z, -1.0, 1.0, ALU.mult, ALU.add)
                    hz = wk.tile([128, 128], F32, tag="hz")
                    nc.gpsimd.tensor_mul(hz, h_G, z)

                    nzb = wk.tile([128, 128], F32, tag="nzb")
                    nc.vector.tensor_mul(nzb, n_, zb)
                    h_new = hp.tile([128, 128], F32, tag="hG")
                    nc.vector.tensor_add(h_new, nzb, hz)
                    h_bf = wk.tile([128, 128], BF16, tag="hbf")
                    nc.vector.tensor_copy(h_bf, h_new)
                    hT_new = hp.tile([128, 128], BF16, tag="hT")
                    for k in range(NK):
                        nc.vector.transpose(
                            hT_new[:, 32 * k:32 * k + 32], h_bf[:, 32 * k:32 * k + 32])
                    if dbg and (not is_dec) and t == 1:
                        p1 = fl.tile([128, 512], F32, tag="p1cp")
                        nc.vector.tensor_copy(p1[:, 0:mw*128], ps[:, 0:mw*128])
                        nc.vector.memset(p1[:, mw*128:512], 0.0)
                        nc.sync.dma_start(d_dbg_w["ps1"].ap(), p1)
                    if dbg and (not is_dec) and t == 0:
                        hcp = wk.tile([128, 128], F32, tag="hTcp")
                        nc.vector.tensor_copy(hcp, hT_new)
                        nc.sync.dma_start(d_dbg_w["hT0"].ap(), hcp)
                        for wn, tl in [("rp", rp), ("r", r), ("t1", t1), ("t2", t2),
                                       ("n", n_), ("zp", zp), ("z", z), ("zb", zb),
                                       ("hz", hz), ("nzb", nzb)]:
                            cp = wk.tile([128, tl.shape[-1] if False else d_dbg_w[wn].shape[1]], F32, tag=f"cp{wn}")
                            nc.vector.tensor_copy(cp, tl)
                            nc.sync.dma_start(d_dbg_w[wn].ap(), cp)
                        gcp = wk.tile([128, 384], F32, tag="gcp0")
                        nc.vector.tensor_copy(gcp, gi)
                        nc.sync.dma_start(d_dbg_w["gi0"].ap(), gcp)
                        pcp = fl.tile([128, 512], F32, tag="pcp0")
                        nc.vector.tensor_copy(pcp[:, 0:mw*128], ps[:, 0:mw*128])
                        nc.vector.memset(pcp[:, mw*128:512], 0.0)
                        nc.sync.dma_start(d_dbg_w["ps0"].ap(), pcp)
                    h_G, hT = h_new, hT_new

                    if dbg and is_dec and t == 0:
                        gicp = wk.tile([128, 384], F32, tag="gicp")
                        nc.vector.tensor_copy(gicp, gi)
                        nc.sync.dma_start(d_dbg_gi.ap(), gicp)
                        pscp = fl.tile([128, 512], F32, tag="pscp")
                        nc.vector.tensor_copy(pscp[:, 0:mw * 128], ps[:, 0:mw * 128])
                        nc.sync.dma_start(d_dbg_ps.ap(), pscp)
                    # psum logits were computed from the PRE-update h, so they
                    # belong to output step t-1 (the reference updates h first).
                    if is_dec and t >= 1:
                        flat = emit_logits(ps, t - 1, flat)

                return flat

            def emit_logits(ps_src, u, flat):
                """Route psum logits (G-layout) for output step u into the
                flat PSUM tile via identity col-tiled matmuls; every 4th
                step DVE-copy the flat tile to the bf16 raw buffer
                (softmax is deferred to the tail)."""
                s = u % 4
                if s == 0:
                    flat = fpp.tile([128, 512], F32, tag="flat")
                lg = wk.tile([128, 128], BF16, tag="lg")
                nc.scalar.copy(lg, ps_src[:, 384:512])
                for g in range(G):
                    nc.tensor.matmul(
                        flat[32 * s:32 * s + 32, 128 * g:128 * g + 128],
                        lhsT=ident[:, 32 * g:32 * g + 32],
                        rhs=lg[:, :],
                        start=True, stop=True,
                        tile_position=(0, 32 * s), skip_group_check=True)
                if s == 3:
                    nc.vector.tensor_copy(raw[:, u // 4, :], flat)
                return flat

            assert tt % 4 == 0
            phase(ss, whh_e, gidx_s, d_tab_e, is_dec=False)
            if dbg:
                nc.sync.dma_start(d_dbg_h.ap(), h_G)
            flat = phase(tt, whh_d, gidx_t, d_tab_d, is_dec=True)
            # logits for the last output step, from the final h
            psf = psp.tile([128, 512], F32, tag="ps")
            for k in range(NK):
                for g in range(G):
                    rhs = whh_d[:, k, :].rearrange(
                        "p (m x) -> p m x", x=512)[:, 3:4, 128 * g:128 * g + 128]
                    out = psf[32 * g:32 * g + 32, 384:512].rearrange(
                        "p (m x) -> p m x", x=128)
                    nc.tensor.matmul(
                        out, lhsT=hT[:, 32 * k:32 * k + 32],
                        rhs=rhs,
                        start=(k == 0), stop=(k == NK - 1),
                        tile_position=(0, 32 * g), skip_group_check=True)
            emit_logits(psf, tt - 1, flat)

            # ---- deferred log-softmax: out = x - ln(sum(exp(x))) ----
            sfo = ctx.enter_context(tc.tile_pool(name="sfo", bufs=2))
            se = singles.tile([128, tt // 4], F32)
            lnse = singles.tile([128, tt // 4], F32)
            for j in range(tt // 4):
                eo = sfo.tile([128, V], F32, tag="eo")
                nc.scalar.activation(eo, raw[:, j, :], AF.Exp,
                                     accum_out=se[:, j:j + 1])
            nc.scalar.activation(lnse, se, AF.Ln)
            for j in range(tt // 4):
                ot = sfo.tile([128, V], F32, tag="ot")
                nc.vector.tensor_scalar_sub(ot, raw[:, j, :], lnse[:, j:j + 1])
                nc.sync.dma_start(d_out.ap()[j], ot)

    if split:
        _split_waits(nc, max_waits=1)
    return nc


# d-permutation of K-chunk k: row 32g+mu of chunk k <-> d = 128g + 32k + mu
_PERM = np.array([[128 * g + 32 * k + mu for g in range(G) for mu in range(32)]
                  for k in range(NK)])  # [NK, 128]


def host_prep(inputs, ss=S, tt=T):
    """Slice/transpose the full inputs into per-core in_maps."""
    f32 = np.float32
    enc_WihT = np.ascontiguousarray(inputs["enc_Wih"].astype(f32).T)   # [D, 3D]
    dec_WihT = np.ascontiguousarray(inputs["dec_Wih"].astype(f32).T)
    enc_WhhT = inputs["enc_Whh"].astype(f32).T                          # [D, 3D]
    dec_WhhT = inputs["dec_Whh"].astype(f32).T
    outWT = inputs["out_W"].astype(f32).T                               # [D, V]

    import ml_dtypes
    bf16 = ml_dtypes.bfloat16
    embT_e = np.ascontiguousarray(inputs["enc_emb"].astype(f32).T
                                  .reshape(NK, 128, V)).astype(bf16)
    embT_d = np.ascontiguousarray(inputs["dec_emb"].astype(f32).T
                                  .reshape(NK, 128, V)).astype(bf16)
    wihT_e = np.ascontiguousarray(enc_WihT.reshape(NK, 128, TD3)).astype(bf16)
    wihT_d = np.ascontiguousarray(dec_WihT.reshape(NK, 128, TD3)).astype(bf16)
    whh_e = np.ascontiguousarray(enc_WhhT[_PERM]).astype(bf16)          # [NK,128,3D]
    whh_d = np.concatenate([dec_WhhT[_PERM], outWT[_PERM]], axis=2)
    whh_d = np.ascontiguousarray(whh_d).astype(bf16)                    # [NK,128,3D+D]

    shared = {
        "embT_enc": embT_e, "embT_dec": embT_d,
        "wihT_enc": wihT_e, "wihT_dec": wihT_d,
        "whh_enc": whh_e, "whh_dec": whh_d,
    }
    shared["ident"] = np.eye(128, dtype=bf16)
    src = np.asarray(inputs["src"])[:, :ss].astype(np.int32)
    trg = np.asarray(inputs["trg"])[:, :tt].astype(np.int32)
    goff = (np.arange(128, dtype=np.int32) // 32 * V)[:, None]
    in_maps = []
    for c in range(NCORES):
        sl = slice(c * BL, (c + 1) * BL)
        m = dict(shared)
        m["gidx_src"] = np.ascontiguousarray(np.tile(src[sl], (G, 1)) + goff)
        m["gidx_trg"] = np.ascontiguousarray(np.tile(trg[sl], (G, 1)) + goff)
        in_maps.append(m)
    return in_maps


_CACHE = {}


def kernel(**inputs) -> np.ndarray:
    nc = _CACHE.get("nc")
    if nc is None:
        nc = build_program()
        _CACHE["nc"] = nc
    in_maps = host_prep(inputs)
    res = run_bass_kernel_spmd(nc, in_maps, core_ids=list(range(NCORES)))
    # unflatten: core out is [T//4, 128, V] with row 32s+b = (b, 4j+s)
    outs = []
    for c in range(NCORES):
        oc = res.results[c]["out"]                       # [ngrp, 128, V]
        oc = oc.reshape(T // 4, 4, BL, V)                # [j, s, b, v]
        outs.append(np.ascontiguousarray(
            oc.transpose(2, 0, 1, 3).reshape(BL, T, V)))
    return np.concatenate(outs, axis=0).astype(np.float32)



# revision 59
# speedup vs baseline: 1.0680x; 1.0680x over previous
"""EncoderDecoderRNN (GRU enc/dec + per-step linear + log_softmax) on 8 trn2 cores.

Data-parallel: batch 256 -> 32 per core. Per core, everything lives in a
"G-layout": a [B=32, D=512] tensor is stored as SBUF [128, 128] with
partition 32*g + b holding d-slice [128g, 128g+128) for sample b. This puts
all 128 partitions to work on elementwise ops (free-dim 128 instead of 512).

Key tricks:
- gi = x @ Wih.T is a table lookup: EmbWih = emb @ Wih.T is [V=512, 1536],
  so per step gi_t[b] = EmbWih[src[b,t]] -- one indirect-DMA gather per step
  from a DRAM table laid out so the gather lands directly in G-layout.
- The recurrent gh = h @ Whh.T runs as 16 matmuls per step: stationary
  h.T chunks [128, 32] at 4 tile-position col-groups (concurrent in the PE
  array), 4 K-chunks PSUM-accumulated, float32r for single-pass fp32 speed.
- Decoder logits (h @ out_W.T) stream as a 4th 128-col block in the same
  matmuls (Whh-stream and outW-stream concatenated in SBUF).
- h.T for the next step's stationary comes from DVE 32x32 block transposes
  (the d-order inside each K-chunk is a fixed permutation, pre-applied to
  the streamed weight rows on the host).
- log_softmax runs on flat [128=4 steps x 32 b, 512] tiles every 4 decoder
  steps: -max via negated reduce, exp with accumulate for the sum, final
  subtract fused into an Identity-activation with per-partition bias.
"""

import numpy as np
from contextlib import ExitStack

import concourse.bass as bass
import concourse.mybir as mybir
import concourse.tile as tile
from concourse.bass_utils import run_bass_kernel_spmd

F32 = mybir.dt.float32
F32R = mybir.dt.float32r
BF16 = mybir.dt.bfloat16
I32 = mybir.dt.int32
AF = mybir.ActivationFunctionType
ALU = mybir.AluOpType

B, S, T, V, D = 256, 256, 256, 512, 512
NCORES = 8
BL = B // NCORES          # 32 samples per core
G = 4                     # d-groups (128 each) on partitions
NK = 4                    # K-chunks of 128 for the D=512 contraction
TD3 = 3 * D               # 1536


def _split_waits(nc, max_waits=1):
    """Walrus in this container accepts at most one sync-wait per instruction;
    split extras into preceding same-engine NoOps."""
    n = 0
    for fn in nc.m.functions:
        for block in fn.blocks:
            out, changed = [], False
            for inst in block.instructions:
                si = inst.sync_info
                waits = list(si.on_wait) if si is not None else []
                if len(waits) > max_waits:
                    changed = True
                    keep = waits[-max_waits:]
                    extra = waits[:-max_waits]
                    for w in extra:
                        n += 1
                        out.append(mybir.InstNoOp(
                            name=f"waitsplit-{n}", engine=inst.engine,
                            ins=[], outs=[],
                            sync_info=mybir.SyncInfo(on_wait=[w], on_update=[])))
                    inst.sync_info = mybir.SyncInfo(
                        on_wait=keep, on_update=list(si.on_update))
                out.append(inst)
            if changed:
                block.instructions = out


def build_program(ss=S, tt=T, split=True, dbg=False):
    """Build the per-core bass program (same on all cores; data differs)."""
    nc = bass.Bass("TRN2", target_bir_lowering=False, debug=False,
                   num_devices=NCORES)

    ngrp = tt // 4
    # ---- DRAM I/O (host-prepped layouts; see host_prep) ----
    # gather indices, pre-offset: gidx[32g+b, t] = g*V + tok[b, t]
    d_gidx_s = nc.dram_tensor("gidx_src", [128, ss], I32, kind="ExternalInput")
    d_gidx_t = nc.dram_tensor("gidx_trg", [128, tt], I32, kind="ExternalInput")
    d_ident = nc.dram_tensor("ident", [128, 128], BF16, kind="ExternalInput")
    # lhsT chunks for the EmbWih builds: embT[c, p, v] = emb[v, 128c+p]
    d_embT_e = nc.dram_tensor("embT_enc", [NK, 128, V], BF16, kind="ExternalInput")
    d_embT_d = nc.dram_tensor("embT_dec", [NK, 128, V], BF16, kind="ExternalInput")
    # rhs chunks: wihT[c, p, q] = Wih[q, 128c+p]
    d_wihT_e = nc.dram_tensor("wihT_enc", [NK, 128, TD3], BF16, kind="ExternalInput")
    d_wihT_d = nc.dram_tensor("wihT_dec", [NK, 128, TD3], BF16, kind="ExternalInput")
    # recurrent streams, Dk-permuted rows; dec has outW columns appended
    d_whh_e = nc.dram_tensor("whh_enc", [NK, 128, TD3], BF16, kind="ExternalInput")
    d_whh_d = nc.dram_tensor("whh_dec", [NK, 128, TD3 + D], BF16, kind="ExternalInput")
    # flat output: out[j, 32s+b, v] = log_softmax(b, 4j+s, v); host reorders
    d_out = nc.dram_tensor("out", [ngrp, 128, V], F32, kind="ExternalOutput")
    # gather tables (device-computed): tab[g*V + v, m*128 + j] = EmbW[v, m*512 + 128g + j]
    d_tab_e = nc.dram_tensor("tab_enc", [G * V, 384], F32, kind="Internal")
    d_tab_d = nc.dram_tensor("tab_dec", [G * V, 384], F32, kind="Internal")
    if dbg:
        d_dbg_h = nc.dram_tensor("dbg_h", [128, 128], F32, kind="ExternalOutput")
        d_dbg_fl = nc.dram_tensor("dbg_fl", [128, 512], F32, kind="ExternalOutput")
        d_dbg_gi = nc.dram_tensor("dbg_gi", [128, 384], F32, kind="ExternalOutput")
        d_dbg_ps = nc.dram_tensor("dbg_ps", [128, 512], F32, kind="ExternalOutput")
        d_dbg_w = {}
        for wn in ["rp", "r", "t1", "t2", "n", "zp", "z", "zb", "hz", "nzb", "gi0", "ps0", "hT0", "ps1"]:
            wid = 384 if wn == "gi0" else (512 if wn in ("ps0", "ps1") else 128)
            d_dbg_w[wn] = nc.dram_tensor(f"dbgw_{wn}", [128, wid], F32, kind="ExternalOutput")

    with tile.TileContext(nc) as tc:
        with ExitStack() as ctx:
            singles = ctx.enter_context(tc.tile_pool(name="singles", bufs=1))

            # ---- persistent SBUF ----
            whh_e = singles.tile([128, NK, TD3], BF16)
            nc.sync.dma_start(whh_e, d_whh_e.ap().rearrange("k p q -> p k q"))
            whh_d = singles.tile([128, NK, TD3 + D], BF16)
            nc.sync.dma_start(whh_d, d_whh_d.ap().rearrange("k p q -> p k q"))

            gidx_s = singles.tile([128, ss], I32)
            nc.sync.dma_start(gidx_s, d_gidx_s.ap())
            gidx_t = singles.tile([128, tt], I32)
            nc.sync.dma_start(gidx_t, d_gidx_t.ap())
            ident = singles.tile([128, 128], BF16)
            nc.sync.dma_start(ident, d_ident.ap())
            # raw logits, flat: raw[32s+b, j, v] = logit(b, 4j+s, v)
            raw = singles.tile([128, tt // 4, V], BF16)

            # ---- build the two gather tables on-device ----
            def build_table(d_embT, d_wihT, d_tab, do_relu):
                with ExitStack() as sctx:
                    setup = sctx.enter_context(tc.tile_pool(name="setup", bufs=1))
                    spsum = sctx.enter_context(
                        tc.tile_pool(name="spsum", bufs=2, space="PSUM"))
                    embT = setup.tile([128, NK, V], BF16)
                    nc.sync.dma_start(embT, d_embT.ap().rearrange("c p v -> p c v"))
                    wihT = setup.tile([128, NK, TD3], BF16)
                    nc.sync.dma_start(wihT, d_wihT.ap().rearrange("c p q -> p c q"))
                    if do_relu:
                        nc.scalar.activation(embT, embT, AF.Relu)
                    for vt in range(V // 128):
                        ps = spsum.tile([128, TD3], F32, tag="embw")
                        for c in range(NK):
                            for nb in range(TD3 // 512):
                                nc.tensor.matmul(
                                    ps[:, 512 * nb:512 * nb + 512],
                                    lhsT=embT[:, c, 128 * vt:128 * vt + 128],
                                    rhs=wihT[:, c, 512 * nb:512 * nb + 512],
                                    start=(c == 0), stop=(c == NK - 1))
                        stage = setup.tile([128, TD3], F32, tag="stage")
                        # cols (m*512 + 128g + j) -> (g, m, j)
                        src = ps[:, :].rearrange("p (m g j) -> p g m j", m=3, g=G)
                        dst = stage[:, :].rearrange("p (g m j) -> p g m j", m=3, g=G)
                        nc.scalar.copy(dst, src)
                        st3 = stage[:, :].rearrange("p (g x) -> p g x", g=G)
                        for g in range(G):
                            nc.sync.dma_start(
                                d_tab.ap()[g * V + 128 * vt: g * V + 128 * vt + 128, :],
                                st3[:, g, :])

            build_table(d_embT_e, d_wihT_e, d_tab_e, do_relu=False)
            build_table(d_embT_d, d_wihT_d, d_tab_d, do_relu=True)

            # ---- loop pools ----
            gip = ctx.enter_context(tc.tile_pool(name="gi", bufs=8))
            psp = ctx.enter_context(tc.tile_pool(name="ps", bufs=4, space="PSUM"))
            fpp = ctx.enter_context(tc.tile_pool(name="fp", bufs=2, space="PSUM"))
            wk = ctx.enter_context(tc.tile_pool(name="wk", bufs=3))
            hp = ctx.enter_context(tc.tile_pool(name="h", bufs=2))
            fl = ctx.enter_context(tc.tile_pool(name="fl", bufs=2))

            h_G = hp.tile([128, 128], F32, tag="hG")
            hT = hp.tile([128, 128], BF16, tag="hT")
            nc.vector.memset(h_G, 0.0)
            nc.vector.memset(hT, 0.0)

            def phase(steps, whh, gidx, tab, is_dec):
                nonlocal h_G, hT
                mw = 4 if is_dec else 3          # streamed 512-blocks per MM
                flat = None
                for t in range(steps):
                    gi = gip.tile([128, 384], F32, tag="gi")
                    nc.gpsimd.indirect_dma_start(
                        out=gi[:, :], out_offset=None, in_=tab.ap(),
                        in_offset=bass.IndirectOffsetOnAxis(ap=gidx[:, t:t + 1], axis=0))
                    ps = psp.tile([128, 512], F32, tag="ps")
                    for k in range(NK):
                        for g in range(G):
                            rhs = whh[:, k, :].rearrange(
                                "p (m x) -> p m x", x=512)[:, 0:mw, 128 * g:128 * g + 128]
                            out = ps[32 * g:32 * g + 32, 0:mw * 128].rearrange(
                                "p (m x) -> p m x", x=128)
                            nc.tensor.matmul(
                                out, lhsT=hT[:, 32 * k:32 * k + 32],
                                rhs=rhs,
                                start=(k == 0), stop=(k == NK - 1),
                                tile_position=(0, 32 * g),
                                skip_group_check=True)
                    # gates; cols [r | z | gh_n (| logits)] of 128 each
                    rp = wk.tile([128, 128], F32, tag="rp")
                    nc.vector.tensor_add(rp, gi[:, 0:128], ps[:, 0:128])
                    r = wk.tile([128, 128], F32, tag="r")
                    nc.scalar.activation(r, rp, AF.Sigmoid)
                    t1 = wk.tile([128, 128], F32, tag="t1")
                    nc.vector.tensor_mul(t1, r, ps[:, 256:384])
                    t2 = wk.tile([128, 128], F32, tag="t2")
                    nc.vector.tensor_add(t2, t1, gi[:, 256:384])
                    n_ = wk.tile([128, 128], F32, tag="n")
                    nc.scalar.activation(n_, t2, AF.Tanh)

                    zp = wk.tile([128, 128], F32, tag="zp")
                    nc.vector.tensor_add(zp, gi[:, 128:256], ps[:, 128:256])
                    z = wk.tile([128, 128], F32, tag="z")
                    nc.scalar.activation(z, zp, AF.Sigmoid)
                    zb = wk.tile([128, 128], F32, tag="zb")
                    nc.vector.tensor_scalar(zb, z, -1.0, 1.0, ALU.mult, ALU.add)
                    hz = wk.tile([128, 128], F32, tag="hz")
                    nc.gpsimd.tensor_mul(hz, h_G, z)

                    nzb = wk.tile([128, 128], F32, tag="nzb")
                    nc.vector.tensor_mul(nzb, n_, zb)
                    h_new = hp.tile([128, 128], F32, tag="hG")
                    nc.vector.tensor_add(h_new, nzb, hz)
                    h_bf = wk.tile([128, 128], BF16, tag="hbf")
                    nc.vector.tensor_copy(h_bf, h_new)
                    hT_new = hp.tile([128, 128], BF16, tag="hT")
                    for k in range(NK):
                        nc.vector.transpose(
                            hT_new[:, 32 * k:32 * k + 32], h_bf[:, 32 * k:32 * k + 32])
                    if dbg and (not is_dec) and t == 1:
                        p1 = fl.tile([128, 512], F32, tag="p1cp")
                        nc.vector.tensor_copy(p1[:, 0:mw*128], ps[:, 0:mw*128])
                        nc.vector.memset(p1[:, mw*128:512], 0.0)
                        nc.sync.dma_start(d_dbg_w["ps1"].ap(), p1)
                    if dbg and (not is_dec) and t == 0:
                        hcp = wk.tile([128, 128], F32, tag="hTcp")
                        nc.vector.tensor_copy(hcp, hT_new)
                        nc.sync.dma_start(d_dbg_w["hT0"].ap(), hcp)
                        for wn, tl in [("rp", rp), ("r", r), ("t1", t1), ("t2", t2),
                                       ("n", n_), ("zp", zp), ("z", z), ("zb", zb),
                                       ("hz", hz), ("nzb", nzb)]:
                            cp = wk.tile([128, tl.shape[-1] if False else d_dbg_w[wn].shape[1]], F32, tag=f"cp{wn}")
                            nc.vector.tensor_copy(cp, tl)
                            nc.sync.dma_start(d_dbg_w[wn].ap(), cp)
                        gcp = wk.tile([128, 384], F32, tag="gcp0")
                        nc.vector.tensor_copy(gcp, gi)
                        nc.sync.dma_start(d_dbg_w["gi0"].ap(), gcp)
                        pcp = fl.tile([128, 512], F32, tag="pcp0")
                        nc.vector.tensor_copy(pcp[:, 0:mw*128], ps[:, 0:mw*128])
                        nc.vector.memset(pcp[:, mw*128:512], 0.0)
                        nc.sync.dma_start(d_dbg_w["ps0"].ap(), pcp)
                    h_G, hT = h_new, hT_new

                    if dbg and is_dec and t == 0:
                        gicp = wk.tile([128, 384], F32, tag="gicp")
                        nc.vector.tensor_copy(gicp, gi)
                        nc.sync.dma_start(d_dbg_gi.ap(), gicp)
                        pscp = fl.tile([128, 512], F32, tag="pscp")
                        nc.vector.tensor_copy(pscp[:, 0:mw * 128], ps[:, 0:mw * 128])
                        nc.sync.dma_start(d_dbg_ps.ap(), pscp)
                    # psum logits were computed from the PRE-update h, so they
                    # belong to output step t-1 (the reference updates h first).
                    if is_dec and t >= 1:
                        flat = emit_logits(ps, t - 1, flat)

                return flat

            def emit_logits(ps_src, u, flat):
                """Route psum logits (G-layout) for output step u into the
                flat PSUM tile via identity col-tiled matmuls; every 4th
                step DVE-copy the flat tile to the bf16 raw buffer
                (softmax is deferred to the tail)."""
                s = u % 4
                if s == 0:
                    flat = fpp.tile([128, 512], F32, tag="flat")
                lg = wk.tile([128, 128], BF16, tag="lg")
                nc.scalar.copy(lg, ps_src[:, 384:512])
                for g in range(G):
                    nc.tensor.matmul(
                        flat[32 * s:32 * s + 32, 128 * g:128 * g + 128],
                        lhsT=ident[:, 32 * g:32 * g + 32],
                        rhs=lg[:, :],
                        start=True, stop=True,
                        tile_position=(0, 32 * s), skip_group_check=True)
                if s == 3:
                    nc.vector.tensor_copy(raw[:, u // 4, :], flat)
                return flat

            assert tt % 4 == 0
            phase(ss, whh_e, gidx_s, d_tab_e, is_dec=False)
            if dbg:
                nc.sync.dma_start(d_dbg_h.ap(), h_G)
            flat = phase(tt, whh_d, gidx_t, d_tab_d, is_dec=True)
            # logits for the last output step, from the final h
            psf = psp.tile([128, 512], F32, tag="ps")
            for k in range(NK):
                for g in range(G):
                    rhs = whh_d[:, k, :].rearrange(
                        "p (m x) -> p m x", x=512)[:, 3:4, 128 * g:128 * g + 128]
                    out = psf[32 * g:32 * g + 32, 384:512].rearrange(
                        "p (m x) -> p m x", x=128)
                    nc.tensor.matmul(
                        out, lhsT=hT[:, 32 * k:32 * k + 32],
                        rhs=rhs,
                        start=(k == 0), stop=(k == NK - 1),
                        tile_position=(0, 32 * g), skip_group_check=True)
            emit_logits(psf, tt - 1, flat)

            # ---- deferred log-softmax: out = x - ln(sum(exp(x))) ----
            # zbias = 0 but data-depends on the final h: fences every exp
            # behind the loop so the scheduler cannot hoist them into the
            # sigmoid/tanh steps (each hoist costs an ACT table reload).
            sfo = ctx.enter_context(tc.tile_pool(name="sfo", bufs=2))
            zbias = singles.tile([128, 1], F32)
            nc.vector.tensor_scalar(zbias, h_G[:, 0:1], 0.0, 0.0,
                                    ALU.mult, ALU.mult)
            se = singles.tile([128, tt // 4], F32)
            lnse = singles.tile([128, tt // 4], F32)
            for j in range(tt // 4):
                eo = sfo.tile([128, V], F32, tag="eo")
                nc.scalar.activation(eo, raw[:, j, :], AF.Exp,
                                     bias=zbias[:, 0:1],
                                     accum_out=se[:, j:j + 1])
            nc.scalar.activation(lnse, se, AF.Ln)
            for j in range(tt // 4):
                ot = sfo.tile([128, V], F32, tag="ot")
                nc.vector.tensor_scalar_sub(ot, raw[:, j, :], lnse[:, j:j + 1])
                nc.sync.dma_start(d_out.ap()[j], ot)

    if split:
        _split_waits(nc, max_waits=1)
    return nc


# d-permutation of K-chunk k: row 32g+mu of chunk k <-> d = 128g + 32k + mu
_PERM = np.array([[128 * g + 32 * k + mu for g in range(G) for mu in range(32)]
                  for k in range(NK)])  # [NK, 128]


def host_prep(inputs, ss=S, tt=T):
    """Slice/transpose the full inputs into per-core in_maps."""
    f32 = np.float32
    enc_WihT = np.ascontiguousarray(inputs["enc_Wih"].astype(f32).T)   # [D, 3D]
    dec_WihT = np.ascontiguousarray(inputs["dec_Wih"].astype(f32).T)
    enc_WhhT = inputs["enc_Whh"].astype(f32).T                          # [D, 3D]
    dec_WhhT = inputs["dec_Whh"].astype(f32).T
    outWT = inputs["out_W"].astype(f32).T                               # [D, V]

    import ml_dtypes
    bf16 = ml_dtypes.bfloat16
    embT_e = np.ascontiguousarray(inputs["enc_emb"].astype(f32).T
                                  .reshape(NK, 128, V)).astype(bf16)
    embT_d = np.ascontiguousarray(inputs["dec_emb"].astype(f32).T
                                  .reshape(NK, 128, V)).astype(bf16)
    wihT_e = np.ascontiguousarray(enc_WihT.reshape(NK, 128, TD3)).astype(bf16)
    wihT_d = np.ascontiguousarray(dec_WihT.reshape(NK, 128, TD3)).astype(bf16)
    whh_e = np.ascontiguousarray(enc_WhhT[_PERM]).astype(bf16)          # [NK,128,3D]
    whh_d = np.concatenate([dec_WhhT[_PERM], outWT[_PERM]], axis=2)
    whh_d = np.ascontiguousarray(whh_d).astype(bf16)                    # [NK,128,3D+D]

    shared = {
        "embT_enc": embT_e, "embT_dec": embT_d,
        "wihT_enc": wihT_e, "wihT_dec": wihT_d,
        "whh_enc": whh_e, "whh_dec": whh_d,
    }
    shared["ident"] = np.eye(128, dtype=bf16)
    src = np.asarray(inputs["src"])[:, :ss].astype(np.int32)
    trg = np.asarray(inputs["trg"])[:, :tt].astype(np.int32)
    goff = (np.arange(128, dtype=np.int32) // 32 * V)[:, None]
    in_maps = []
    for c in range(NCORES):
        sl = slice(c * BL, (c + 1) * BL)
        m = dict(shared)
        m["gidx_src"] = np.ascontiguousarray(np.tile(src[sl], (G, 1)) + goff)
        m["gidx_trg"] = np.ascontiguousarray(np.tile(trg[sl], (G, 1)) + goff)
        in_maps.append(m)
    return in_maps


_CACHE = {}


def kernel(**inputs) -> np.ndarray:
    nc = _CACHE.get("nc")
    if nc is None:
        nc = build_program()
        _CACHE["nc"] = nc
    in_maps = host_prep(inputs)
    res = run_bass_kernel_spmd(nc, in_maps, core_ids=list(range(NCORES)))
    # unflatten: core out is [T//4, 128, V] with row 32s+b = (b, 4j+s)
    outs = []
    for c in range(NCORES):
        oc = res.results[c]["out"]                       # [ngrp, 128, V]
        oc = oc.reshape(T // 4, 4, BL, V)                # [j, s, b, v]
        outs.append(np.ascontiguousarray(
            oc.transpose(2, 0, 1, 3).reshape(BL, T, V)))
    return np.concatenate(outs, axis=0).astype(np.float32)

